# revision 2
# baseline (speedup 1.0000x reference)
"""DeepseekMoE Trainium2 kernel (data-parallel dense, 8 NeuronCores).

Strategy: shard tokens (N=8192) across the 8 cores (1024 tokens each).
Each core computes on-device: the gate (true-fp32 matmul so top-2 selection
matches the fp32 reference bit-for-bit), all 8 routed experts + the shared
expert (f32r matmuls), and the weighted top-2 combine (DMA-accumulate).
BatchNorm (eval mode) is folded into the expert weights host-side — a pure
parameter transformation; all per-token math runs on device.

Host responsibilities: slicing/transposing inputs, weight re-tiling,
concatenating per-core outputs. No per-token arithmetic on host.
"""
import numpy as np
import concourse.bass as bass
import concourse.mybir as mybir
import concourse.tile as tile
from concourse import bacc
from concourse.bass_utils import run_bass_kernel_spmd
from concourse.masks import make_identity

F32 = mybir.dt.float32
F32R = mybir.dt.float32r

N_TOKENS, D, H, O, E = 8192, 1024, 2048, 1024, 8
KD, KH, MH, MO = D // 128, H // 128, H // 128, O // 128
NEXP = 9  # 8 routed + shared (index 8)
EPS = 1e-5
BIG = 1e30
N_CORES = 8
TOK = N_TOKENS // N_CORES


def _fold_params(inp):
    """Fold eval-mode BN into the expert weights (host-side, O(weights))."""
    def tiles_kxm(V, KT, MT):
        return np.ascontiguousarray(
            V.reshape(KT, 128, MT, 128).transpose(2, 1, 0, 3))

    V1s, V2s, V3s, C1s, C2s, C3s = [], [], [], [], [], []
    for e in range(NEXP):
        if e < E:
            W1, b1 = inp['W1'][e], inp['b1'][e]
            g1, be1, m1, v1 = inp['g1'][e], inp['be1'][e], inp['m1'][e], inp['v1'][e]
            W2, b2 = inp['W2'][e], inp['b2'][e]
            g2, be2, m2, v2 = inp['g2'][e], inp['be2'][e], inp['m2'][e], inp['v2'][e]
            W3, b3 = inp['W3'][e], inp['b3'][e]
        else:
            W1, b1 = inp['sW1'], inp['sb1']
            g1, be1, m1, v1 = inp['sg1'], inp['sbe1'], inp['sm1'], inp['sv1']
            W2, b2 = inp['sW2'], inp['sb2']
            g2, be2, m2, v2 = inp['sg2'], inp['sbe2'], inp['sm2'], inp['sv2']
            W3, b3 = inp['sW3'], inp['sb3']
        s1 = g1 / np.sqrt(v1 + EPS); t1 = be1 - m1 * s1
        s2 = g2 / np.sqrt(v2 + EPS); t2 = be2 - m2 * s2
        V1 = W1.T.astype(np.float32)
        c1 = b1.astype(np.float32)
        V2 = (s1[:, None] * W2.T).astype(np.float32)
        c2 = (b2 + t1 @ W2.T).astype(np.float32)
        V3 = (s2[:, None] * W3.T).astype(np.float32)
        c3 = (b3 + t2 @ W3.T).astype(np.float32)
        V1s.append(tiles_kxm(V1, KD, MH))
        V2s.append(tiles_kxm(V2, KH, MH))
        V3s.append(tiles_kxm(V3, KH, MO))
        C1s.append(np.ascontiguousarray(c1.reshape(MH, 128).T))
        C2s.append(np.ascontiguousarray(c2.reshape(MH, 128).T))
        C3s.append(np.ascontiguousarray(c3.reshape(MO, 128).T))
    return (np.stack(V1s), np.stack(V2s), np.stack(V3s),
            np.stack(C1s), np.stack(C2s), np.stack(C3s))


def _build_dense_kernel():
    nc = bacc.Bacc("TRN2", target_bir_lowering=False, debug=False,
                   num_devices=N_CORES)

    xTr_d = nc.dram_tensor("xTr", [D, TOK], F32R, kind="ExternalInput")
    xT32_d = nc.dram_tensor("xT32", [D, TOK], F32, kind="ExternalInput")
    wg_d = nc.dram_tensor("WgT", [D, E], F32, kind="ExternalInput")
    V1_d = nc.dram_tensor("V1r", [NEXP, MH, 128, KD, 128], F32R, kind="ExternalInput")
    V2_d = nc.dram_tensor("V2r", [NEXP, MH, 128, KH, 128], F32R, kind="ExternalInput")
    V3_d = nc.dram_tensor("V3r", [NEXP, MO, 128, KH, 128], F32R, kind="ExternalInput")
    C1_d = nc.dram_tensor("C1", [NEXP, 128, MH], F32, kind="ExternalInput")
    C2_d = nc.dram_tensor("C2", [NEXP, 128, MH], F32, kind="ExternalInput")
    C3_d = nc.dram_tensor("C3", [NEXP, 128, MO], F32, kind="ExternalInput")
    out_d = nc.dram_tensor("out", [O, TOK], F32, kind="ExternalOutput")

    NT = TOK // 512
    TT = TOK // 128
    Relu = mybir.ActivationFunctionType.Relu
    Sigm = mybir.ActivationFunctionType.Sigmoid
    Expf = mybir.ActivationFunctionType.Exp

    with tile.TileContext(nc) as tc:
        with tc.tile_pool(name="const", bufs=1) as cpool, \
             tc.tile_pool(name="acts", bufs=1) as apool, \
             tc.tile_pool(name="wts", bufs=3) as wpool, \
             tc.tile_pool(name="bias", bufs=2) as bpool, \
             tc.tile_pool(name="wb", bufs=1) as wbpool, \
             tc.tile_pool(name="tmp", bufs=3) as tpool, \
             tc.tile_pool(name="gate", bufs=2) as gpool, \
             tc.tile_pool(name="ps", bufs=8, space="PSUM") as ps, \
             tc.tile_pool(name="dram", bufs=1, space="DRAM") as dpool:

            xTr_sb = cpool.tile([128, KD, TOK], F32R)
            nc.sync.dma_start(xTr_sb, xTr_d.ap().rearrange("(k p) t -> p k t", p=128))
            wg_sb = cpool.tile([128, KD, E], F32)
            nc.sync.dma_start(wg_sb, wg_d.ap().rearrange("(k p) e -> p k e", p=128))
            ident = cpool.tile([128, 128], F32)
            make_identity(nc, ident)

            a1 = apool.tile([128, KH, TOK], F32R)
            # xT32 is gate-only; its slot is reused by a2 (experts-only)
            xT32_sb = apool.tile([128, KD, TOK], F32, tag="xa2", name="xT32_sb")
            nc.sync.dma_start(xT32_sb, xT32_d.ap().rearrange("(k p) t -> p k t", p=128))
            wsumT = cpool.tile([E, TOK], F32)

            # ---- gate (true fp32): top-2 softmax combine weights ----
            for ti in range(TT):
                tsl = slice(ti * 128, (ti + 1) * 128)
                pg = ps.tile([128, 512], F32, tag="ps", name="pg")[:, :E]
                for ki in range(KD):
                    nc.tensor.matmul(pg, xT32_sb[:, ki, tsl], wg_sb[:, ki],
                                     start=(ki == 0), stop=(ki == KD - 1))
                s = gpool.tile([128, E], F32)
                nc.vector.tensor_copy(s, pg)
                m1 = gpool.tile([128, 1], F32)
                nc.vector.tensor_reduce(m1, s, axis=mybir.AxisListType.X,
                                        op=mybir.AluOpType.max)
                nm1 = gpool.tile([128, 1], F32)
                nc.vector.tensor_scalar_mul(nm1, m1, -1.0)
                msk = gpool.tile([128, E], F32)
                nc.vector.tensor_tensor(msk, s, m1.to_broadcast((128, E)),
                                        op=mybir.AluOpType.is_equal)
                nc.vector.tensor_scalar_mul(msk, msk, -BIG)
                nc.vector.tensor_tensor(msk, s, msk, op=mybir.AluOpType.add)
                m2 = gpool.tile([128, 1], F32)
                nc.vector.tensor_reduce(m2, msk, axis=mybir.AxisListType.X,
                                        op=mybir.AluOpType.max)
                r = gpool.tile([128, E], F32)
                nc.scalar.activation(r, s, Expf, bias=nm1, scale=1.0)
                e2 = gpool.tile([128, 1], F32)
                nc.scalar.activation(e2, m2, Expf, bias=nm1, scale=1.0)
                den = gpool.tile([128, 1], F32)
                nc.vector.tensor_scalar_add(den, e2, 1.0)
                rec = gpool.tile([128, 1], F32)
                nc.vector.reciprocal(rec, den)
                ge = gpool.tile([128, E], F32)
                nc.vector.tensor_tensor(ge, s, m2.to_broadcast((128, E)),
                                        op=mybir.AluOpType.is_ge)
                w = gpool.tile([128, E], F32)
                nc.vector.tensor_tensor(w, r, ge, op=mybir.AluOpType.mult)
                nc.vector.tensor_scalar_mul(w, w, rec)
                pt = ps.tile([128, 512], F32, tag="ps", name="pt")[:, :128]
                nc.tensor.transpose(pt[:E], w, ident)
                nc.vector.tensor_copy(wsumT[:, tsl], pt[:E])

            wsumT_dram = dpool.tile([E, TOK], F32)
            nc.sync.dma_start(wsumT_dram, wsumT)

            # a2 reuses the gate's xT32 slot (gate fully precedes expert L2)
            a2 = apool.tile([128, KH, TOK], F32R, tag="xa2", name="a2")

            # ---- experts: 8 routed + shared ----
            for e in range(NEXP):
                c1_sb = bpool.tile([128, MH], F32, tag="c12")
                nc.sync.dma_start(c1_sb, C1_d.ap()[e])
                c2_sb = bpool.tile([128, MH], F32, tag="c12")
                nc.sync.dma_start(c2_sb, C2_d.ap()[e])
                c3_sb = bpool.tile([128, MO], F32, tag="c3")
                nc.sync.dma_start(c3_sb, C3_d.ap()[e])
                if e < E:
                    wbc = wbpool.tile([128, TOK], F32)
                    nc.sync.dma_start(
                        wbc, wsumT_dram[e][None, :].to_broadcast((128, TOK)))

                for mi in range(MH):
                    wt = wpool.tile([128, KD, 128], F32R, tag="w")
                    nc.sync.dma_start(wt, V1_d.ap()[e, mi])
                    for ni in range(NT):
                        nsl = slice(ni * 512, (ni + 1) * 512)
                        pp = ps.tile([128, 512], F32, tag="ps", name="pp1")
                        for ki in range(KD):
                            nc.tensor.matmul(pp, wt[:, ki], xTr_sb[:, ki, nsl],
                                             start=(ki == 0), stop=(ki == KD - 1))
                        nc.scalar.activation(a1[:, mi, nsl], pp, Relu,
                                             bias=c1_sb[:, mi:mi + 1], scale=1.0)
                for mi in range(MH):
                    wta = wpool.tile([128, KD, 128], F32R, tag="w", name="wta")
                    nc.sync.dma_start(wta, V2_d.ap()[e, mi, :, :KD])
                    wtb = wpool.tile([128, KD, 128], F32R, tag="w", name="wtb")
                    nc.sync.dma_start(wtb, V2_d.ap()[e, mi, :, KD:])
                    for ni in range(NT):
                        nsl = slice(ni * 512, (ni + 1) * 512)
                        pp = ps.tile([128, 512], F32, tag="ps", name="pp2")
                        for ki in range(KH):
                            wt = wta if ki < KD else wtb
                            nc.tensor.matmul(pp, wt[:, ki % KD], a1[:, ki, nsl],
                                             start=(ki == 0), stop=(ki == KH - 1))
                        nc.scalar.activation(a2[:, mi, nsl], pp, Relu,
                                             bias=c2_sb[:, mi:mi + 1], scale=1.0)
                for mi in range(MO):
                    wta = wpool.tile([128, KD, 128], F32R, tag="w", name="wta3")
                    nc.sync.dma_start(wta, V3_d.ap()[e, mi, :, :KD])
                    wtb = wpool.tile([128, KD, 128], F32R, tag="w", name="wtb3")
                    nc.sync.dma_start(wtb, V3_d.ap()[e, mi, :, KD:])
                    for ni in range(NT):
                        nsl = slice(ni * 512, (ni + 1) * 512)
                        pp = ps.tile([128, 512], F32, tag="ps", name="pp3")
                        for ki in range(KH):
                            wt = wta if ki < KD else wtb
                            nc.tensor.matmul(pp, wt[:, ki % KD], a2[:, ki, nsl],
                                             start=(ki == 0), stop=(ki == KH - 1))
                        sg = tpool.tile([128, 512], F32)
                        nc.scalar.activation(sg, pp, Sigm,
                                             bias=c3_sb[:, mi:mi + 1], scale=1.0)
                        osl = out_d.ap()[mi * 128:(mi + 1) * 128, nsl]
                        if e == 0:
                            nc.vector.tensor_tensor(sg, sg, wbc[:, nsl],
                                                    op=mybir.AluOpType.mult)
                            nc.sync.dma_start(osl, sg)
                        elif e < E:
                            nc.vector.tensor_tensor(sg, sg, wbc[:, nsl],
                                                    op=mybir.AluOpType.mult)
                            nc.gpsimd.dma_start(osl, sg,
                                                accum_op=mybir.AluOpType.add)
                        else:
                            nc.gpsimd.dma_start(osl, sg,
                                                accum_op=mybir.AluOpType.add)

    nc.compile()
    return nc


_CACHED = {}


def kernel(**inputs) -> np.ndarray:
    inp = {k: np.asarray(v) for k, v in inputs.items()}
    if "nc" not in _CACHED:
        _CACHED["nc"] = _build_dense_kernel()
    nc = _CACHED["nc"]

    V1r, V2r, V3r, C1, C2, C3 = _fold_params(inp)
    shared = dict(V1r=V1r, V2r=V2r, V3r=V3r, C1=C1, C2=C2, C3=C3,
                  WgT=np.ascontiguousarray(inp['Wg'].T.astype(np.float32)))
    x = inp['x'].astype(np.float32)
    maps = []
    for c in range(N_CORES):
        xT = np.ascontiguousarray(x[c * TOK:(c + 1) * TOK].T)
        m = dict(shared)
        m['xTr'] = xT
        m['xT32'] = xT
        maps.append(m)

    _CACHED["timing"] = [(nc, maps)]
    res = run_bass_kernel_spmd(nc, maps, core_ids=list(range(N_CORES)))
    out = np.concatenate([r["out"].T for r in res.results], axis=0)
    return out.astype(np.float32)


# revision 3
# speedup vs baseline: 101.0162x; 101.0162x over previous
"""DeepseekMoE Trainium2 kernel — routed 3-stage pipeline on 8 NeuronCores.

Stage A (data-parallel, 1024 tokens/core): gate computed with a true-fp32
  matmul (so top-2 selection matches the fp32 reference) producing the
  normalized top-2 combine weights, plus the shared-expert FFN.
Stage B (expert-parallel, one expert per core): 3-layer FFN over the tokens
  routed to that expert (host-gathered to a runtime-sized capacity), with
  the per-token combine weight applied on device.
Stage C (data-parallel): out = shared + contrib1 + contrib2 on device.

Expert matmuls run in float32r (fp22 multiply, fp32 accumulate). Eval-mode
BatchNorm is folded into the expert weights host-side (pure parameter
preprocessing). Host code between stages only moves data (gather/scatter by
the device-computed top-2 indices); all per-token arithmetic is on device.
"""
import numpy as np
import concourse.mybir as mybir
import concourse.tile as tile
from concourse import bacc
from concourse.bass_utils import run_bass_kernel_spmd

F32 = mybir.dt.float32
F32R = mybir.dt.float32r

N_TOKENS, D, H, O, E = 8192, 1024, 2048, 1024, 8
KD, KH, MH, MO = D // 128, H // 128, H // 128, O // 128
NEXP = 9  # 8 routed experts + shared (index 8)
EPS = 1e-5
BIG = 1e30
N_CORES = 8
TOK = N_TOKENS // N_CORES
Relu = mybir.ActivationFunctionType.Relu
Sigm = mybir.ActivationFunctionType.Sigmoid
Expf = mybir.ActivationFunctionType.Exp


# ---------------------------------------------------------------- host prep
def _fold_params(inp):
    """Fold eval-mode BN into the expert weights (host-side, O(weights))."""
    def tiles_kxm(V, KT, MT):
        return np.ascontiguousarray(
            V.reshape(KT, 128, MT, 128).transpose(2, 1, 0, 3))

    V1s, V2s, V3s, C1s, C2s, C3s = [], [], [], [], [], []
    for e in range(NEXP):
        if e < E:
            W1, b1 = inp['W1'][e], inp['b1'][e]
            g1, be1, m1, v1 = inp['g1'][e], inp['be1'][e], inp['m1'][e], inp['v1'][e]
            W2, b2 = inp['W2'][e], inp['b2'][e]
            g2, be2, m2, v2 = inp['g2'][e], inp['be2'][e], inp['m2'][e], inp['v2'][e]
            W3, b3 = inp['W3'][e], inp['b3'][e]
        else:
            W1, b1 = inp['sW1'], inp['sb1']
            g1, be1, m1, v1 = inp['sg1'], inp['sbe1'], inp['sm1'], inp['sv1']
            W2, b2 = inp['sW2'], inp['sb2']
            g2, be2, m2, v2 = inp['sg2'], inp['sbe2'], inp['sm2'], inp['sv2']
            W3, b3 = inp['sW3'], inp['sb3']
        s1 = g1 / np.sqrt(v1 + EPS); t1 = be1 - m1 * s1
        s2 = g2 / np.sqrt(v2 + EPS); t2 = be2 - m2 * s2
        V1 = W1.T.astype(np.float32)
        c1 = b1.astype(np.float32)
        V2 = (s1[:, None] * W2.T).astype(np.float32)
        c2 = (b2 + t1 @ W2.T).astype(np.float32)
        V3 = (s2[:, None] * W3.T).astype(np.float32)
        c3 = (b3 + t2 @ W3.T).astype(np.float32)
        V1s.append(tiles_kxm(V1, KD, MH))
        V2s.append(tiles_kxm(V2, KH, MH))
        V3s.append(tiles_kxm(V3, KH, MO))
        C1s.append(np.ascontiguousarray(c1.reshape(MH, 128).T))
        C2s.append(np.ascontiguousarray(c2.reshape(MH, 128).T))
        C3s.append(np.ascontiguousarray(c3.reshape(MO, 128).T))
    return (np.stack(V1s), np.stack(V2s), np.stack(V3s),
            np.stack(C1s), np.stack(C2s), np.stack(C3s))


# ------------------------------------------------------------ kernel builders
def _ffn3(nc, pools, xg, V1_ap, V2_ap, V3_ap, c1_sb, c2_sb, c3_sb, ntok, emit):
    """Feature-major 3-layer FFN on `ntok` tokens (multiple of 128).
    xg: SBUF [128, KD, ntok] f32r. emit(mi, nsl, psum) consumes L3 psum."""
    wpool, ps, apool = pools["w"], pools["ps"], pools["act"]
    nsls = [slice(s, min(s + 512, ntok)) for s in range(0, ntok, 512)]
    a1 = apool.tile([128, KH, ntok], F32R, tag="a1", name="a1")
    for mi in range(MH):
        wt = wpool.tile([128, KD, 128], F32R, tag="w", name="wt1")
        nc.sync.dma_start(wt, V1_ap[mi])
        for nsl in nsls:
            nn = nsl.stop - nsl.start
            pp = ps.tile([128, 512], F32, tag="ps", name="pp1")[:, :nn]
            for ki in range(KD):
                nc.tensor.matmul(pp, wt[:, ki], xg[:, ki, nsl],
                                 start=(ki == 0), stop=(ki == KD - 1))
            nc.scalar.activation(a1[:, mi, nsl], pp, Relu,
                                 bias=c1_sb[:, mi:mi + 1], scale=1.0)
    a2 = apool.tile([128, KH, ntok], F32R, tag="a2", name="a2")
    for mi in range(MH):
        wta = wpool.tile([128, KD, 128], F32R, tag="w", name="wta")
        nc.sync.dma_start(wta, V2_ap[mi, :, :KD])
        wtb = wpool.tile([128, KD, 128], F32R, tag="w", name="wtb")
        nc.sync.dma_start(wtb, V2_ap[mi, :, KD:])
        for nsl in nsls:
            nn = nsl.stop - nsl.start
            pp = ps.tile([128, 512], F32, tag="ps", name="pp2")[:, :nn]
            for ki in range(KH):
                wt = wta if ki < KD else wtb
                nc.tensor.matmul(pp, wt[:, ki % KD], a1[:, ki, nsl],
                                 start=(ki == 0), stop=(ki == KH - 1))
            nc.scalar.activation(a2[:, mi, nsl], pp, Relu,
                                 bias=c2_sb[:, mi:mi + 1], scale=1.0)
    for mi in range(MO):
        wta = wpool.tile([128, KD, 128], F32R, tag="w", name="wta3")
        nc.sync.dma_start(wta, V3_ap[mi, :, :KD])
        wtb = wpool.tile([128, KD, 128], F32R, tag="w", name="wtb3")
        nc.sync.dma_start(wtb, V3_ap[mi, :, KD:])
        for nsl in nsls:
            nn = nsl.stop - nsl.start
            pp = ps.tile([128, 512], F32, tag="ps", name="pp3")[:, :nn]
            for ki in range(KH):
                wt = wta if ki < KD else wtb
                nc.tensor.matmul(pp, wt[:, ki % KD], a2[:, ki, nsl],
                                 start=(ki == 0), stop=(ki == KH - 1))
            emit(mi, nsl, pp)


def _build_kernel_A():
    """Gate (true fp32) + shared expert. Outputs wsum [TOK, E], shared [O, TOK]."""
    nc = bacc.Bacc("TRN2", target_bir_lowering=False, debug=False,
                   num_devices=N_CORES)
    xTr_d = nc.dram_tensor("xTr", [D, TOK], F32R, kind="ExternalInput")
    xT32_d = nc.dram_tensor("xT32", [D, TOK], F32, kind="ExternalInput")
    wg_d = nc.dram_tensor("WgT", [D, E], F32, kind="ExternalInput")
    V1_d = nc.dram_tensor("V1s", [MH, 128, KD, 128], F32R, kind="ExternalInput")
    V2_d = nc.dram_tensor("V2s", [MH, 128, KH, 128], F32R, kind="ExternalInput")
    V3_d = nc.dram_tensor("V3s", [MO, 128, KH, 128], F32R, kind="ExternalInput")
    C1_d = nc.dram_tensor("C1s", [128, MH], F32, kind="ExternalInput")
    C2_d = nc.dram_tensor("C2s", [128, MH], F32, kind="ExternalInput")
    C3_d = nc.dram_tensor("C3s", [128, MO], F32, kind="ExternalInput")
    wsum_d = nc.dram_tensor("wsum", [TOK, E], F32, kind="ExternalOutput")
    sh_d = nc.dram_tensor("shared", [O, TOK], F32, kind="ExternalOutput")

    TT = TOK // 128
    with tile.TileContext(nc) as tc:
        with tc.tile_pool(name="const", bufs=1) as cpool, \
             tc.tile_pool(name="acts", bufs=1) as apool, \
             tc.tile_pool(name="wts", bufs=3) as wpool, \
             tc.tile_pool(name="bias", bufs=1) as bpool, \
             tc.tile_pool(name="tmp", bufs=3) as tpool, \
             tc.tile_pool(name="gate", bufs=2) as gpool, \
             tc.tile_pool(name="ps", bufs=8, space="PSUM") as ps:
            xTr_sb = cpool.tile([128, KD, TOK], F32R)
            nc.sync.dma_start(xTr_sb, xTr_d.ap().rearrange("(k p) t -> p k t", p=128))
            wg_sb = cpool.tile([128, KD, E], F32)
            nc.sync.dma_start(wg_sb, wg_d.ap().rearrange("(k p) e -> p k e", p=128))
            # xT32 (gate-only) shares its slot with a2 (FFN L2+)
            xT32_sb = apool.tile([128, KD, TOK], F32, tag="a2", name="xT32_sb")
            nc.sync.dma_start(xT32_sb, xT32_d.ap().rearrange("(k p) t -> p k t", p=128))

            for ti in range(TT):
                tsl = slice(ti * 128, (ti + 1) * 128)
                pg = ps.tile([128, 512], F32, tag="ps", name="pg")[:, :E]
                for ki in range(KD):
                    nc.tensor.matmul(pg, xT32_sb[:, ki, tsl], wg_sb[:, ki],
                                     start=(ki == 0), stop=(ki == KD - 1))
                s = gpool.tile([128, E], F32)
                nc.vector.tensor_copy(s, pg)
                m1 = gpool.tile([128, 1], F32)
                nc.vector.tensor_reduce(m1, s, axis=mybir.AxisListType.X,
                                        op=mybir.AluOpType.max)
                nm1 = gpool.tile([128, 1], F32)
                nc.vector.tensor_scalar_mul(nm1, m1, -1.0)
                msk = gpool.tile([128, E], F32)
                nc.vector.tensor_tensor(msk, s, m1.to_broadcast((128, E)),
                                        op=mybir.AluOpType.is_equal)
                nc.vector.tensor_scalar_mul(msk, msk, -BIG)
                nc.vector.tensor_tensor(msk, s, msk, op=mybir.AluOpType.add)
                m2 = gpool.tile([128, 1], F32)
                nc.vector.tensor_reduce(m2, msk, axis=mybir.AxisListType.X,
                                        op=mybir.AluOpType.max)
                r = gpool.tile([128, E], F32)
                nc.scalar.activation(r, s, Expf, bias=nm1, scale=1.0)
                e2 = gpool.tile([128, 1], F32)
                nc.scalar.activation(e2, m2, Expf, bias=nm1, scale=1.0)
                den = gpool.tile([128, 1], F32)
                nc.vector.tensor_scalar_add(den, e2, 1.0)
                rec = gpool.tile([128, 1], F32)
                nc.vector.reciprocal(rec, den)
                ge = gpool.tile([128, E], F32)
                nc.vector.tensor_tensor(ge, s, m2.to_broadcast((128, E)),
                                        op=mybir.AluOpType.is_ge)
                w = gpool.tile([128, E], F32)
                nc.vector.tensor_tensor(w, r, ge, op=mybir.AluOpType.mult)
                nc.vector.tensor_scalar_mul(w, w, rec)
                nc.sync.dma_start(wsum_d.ap()[tsl], w)

            c1_sb = bpool.tile([128, MH], F32, name="c1_sb")
            nc.sync.dma_start(c1_sb, C1_d.ap())
            c2_sb = bpool.tile([128, MH], F32, name="c2_sb")
            nc.sync.dma_start(c2_sb, C2_d.ap())
            c3_sb = bpool.tile([128, MO], F32, name="c3_sb")
            nc.sync.dma_start(c3_sb, C3_d.ap())

            def emit(mi, nsl, pp):
                nn = nsl.stop - nsl.start
                sg = tpool.tile([128, 512], F32, name="sg")[:, :nn]
                nc.scalar.activation(sg, pp, Sigm,
                                     bias=c3_sb[:, mi:mi + 1], scale=1.0)
                nc.sync.dma_start(sh_d.ap()[mi * 128:(mi + 1) * 128, nsl], sg)

            pools = {"w": wpool, "ps": ps, "act": apool}
            _ffn3(nc, pools, xTr_sb, V1_d.ap(), V2_d.ap(), V3_d.ap(),
                  c1_sb, c2_sb, c3_sb, TOK, emit)
    nc.compile()
    return nc


def _build_kernel_B(chunks):
    """One expert per core on gathered tokens; output pre-weighted [O, cap]."""
    C = sum(chunks)
    nc = bacc.Bacc("TRN2", target_bir_lowering=False, debug=False,
                   num_devices=N_CORES)
    xg_d = nc.dram_tensor("xg", [D, C], F32R, kind="ExternalInput")
    wrow_d = nc.dram_tensor("wrow", [C], F32, kind="ExternalInput")
    V1_d = nc.dram_tensor("V1s", [MH, 128, KD, 128], F32R, kind="ExternalInput")
    V2_d = nc.dram_tensor("V2s", [MH, 128, KH, 128], F32R, kind="ExternalInput")
    V3_d = nc.dram_tensor("V3s", [MO, 128, KH, 128], F32R, kind="ExternalInput")
    C1_d = nc.dram_tensor("C1s", [128, MH], F32, kind="ExternalInput")
    C2_d = nc.dram_tensor("C2s", [128, MH], F32, kind="ExternalInput")
    C3_d = nc.dram_tensor("C3s", [128, MO], F32, kind="ExternalInput")
    outb_d = nc.dram_tensor("outb", [O, C], F32, kind="ExternalOutput")

    with tile.TileContext(nc) as tc:
        with tc.tile_pool(name="xgp", bufs=1) as xgpool, \
             tc.tile_pool(name="acts", bufs=1) as apool, \
             tc.tile_pool(name="wts", bufs=4) as wpool, \
             tc.tile_pool(name="bias", bufs=1) as bpool, \
             tc.tile_pool(name="wb", bufs=1) as wbpool, \
             tc.tile_pool(name="tmp", bufs=3) as tpool, \
             tc.tile_pool(name="ps", bufs=8, space="PSUM") as ps:
            c1_sb = bpool.tile([128, MH], F32, name="c1_sb")
            nc.sync.dma_start(c1_sb, C1_d.ap())
            c2_sb = bpool.tile([128, MH], F32, name="c2_sb")
            nc.sync.dma_start(c2_sb, C2_d.ap())
            c3_sb = bpool.tile([128, MO], F32, name="c3_sb")
            nc.sync.dma_start(c3_sb, C3_d.ap())
            pools = {"w": wpool, "ps": ps, "act": apool}

            off = 0
            mx = max(chunks)
            for ch in chunks:
                xg = xgpool.tile([128, KD, mx], F32R, tag="xg", name="xg")
                nc.sync.dma_start(
                    xg[:, :, :ch],
                    xg_d.ap().rearrange("(k p) t -> p k t", p=128)[:, :, off:off + ch])
                wbc = wbpool.tile([128, mx], F32, tag="wbc", name="wbc")
                nc.sync.dma_start(
                    wbc[:, :ch],
                    wrow_d.ap()[None, off:off + ch].to_broadcast((128, ch)))

                def emit(mi, nsl, pp, off=off, wbc=wbc):
                    nn = nsl.stop - nsl.start
                    sg = tpool.tile([128, 512], F32, name="sg")[:, :nn]
                    nc.scalar.activation(sg, pp, Sigm,
                                         bias=c3_sb[:, mi:mi + 1], scale=1.0)
                    nc.vector.tensor_tensor(sg, sg, wbc[:, nsl],
                                            op=mybir.AluOpType.mult)
                    nc.sync.dma_start(
                        outb_d.ap()[mi * 128:(mi + 1) * 128,
                                    off + nsl.start:off + nsl.stop], sg)

                _ffn3(nc, pools, xg[:, :, :ch], V1_d.ap(), V2_d.ap(),
                      V3_d.ap(), c1_sb, c2_sb, c3_sb, ch, emit)
                off += ch
    nc.compile()
    return nc


def _build_kernel_C():
    """out = sharedT + cont1 + cont2, all token-major [TOK, O]."""
    nc = bacc.Bacc("TRN2", target_bir_lowering=False, debug=False,
                   num_devices=N_CORES)
    sh_d = nc.dram_tensor("sharedT", [TOK, O], F32, kind="ExternalInput")
    c1_d = nc.dram_tensor("cont1", [TOK, O], F32, kind="ExternalInput")
    c2_d = nc.dram_tensor("cont2", [TOK, O], F32, kind="ExternalInput")
    out_d = nc.dram_tensor("out", [TOK, O], F32, kind="ExternalOutput")
    with tile.TileContext(nc) as tc:
        with tc.tile_pool(name="sb", bufs=3) as sb:
            for ti in range(TOK // 128):
                tsl = slice(ti * 128, (ti + 1) * 128)
                a = sb.tile([128, O], F32, name="a")
                nc.sync.dma_start(a, sh_d.ap()[tsl])
                b = sb.tile([128, O], F32, name="b")
                nc.sync.dma_start(b, c1_d.ap()[tsl])
                c = sb.tile([128, O], F32, name="c")
                nc.sync.dma_start(c, c2_d.ap()[tsl])
                nc.vector.tensor_tensor(a, a, b, op=mybir.AluOpType.add)
                nc.vector.tensor_tensor(a, a, c, op=mybir.AluOpType.add)
                nc.sync.dma_start(out_d.ap()[tsl], a)
    nc.compile()
    return nc


# ------------------------------------------------------------------ host glue
def _route_from_wsum(wsum):
    """Top-2 experts per token from the device-computed combine weights."""
    n = wsum.shape[0]
    top2 = np.argpartition(-wsum, 2, axis=1)[:, :2]
    sel = np.zeros_like(wsum, dtype=bool)
    sel[np.arange(n)[:, None], top2] = True
    idx = [np.nonzero(sel[:, e])[0] for e in range(E)]
    counts = np.array([len(i) for i in idx])
    cap = int(np.ceil(counts.max() / 128) * 128)
    n_chunks = max(1, -(-cap // 1152))
    base = cap // n_chunks // 128 * 128
    chunks = [base] * n_chunks
    for i in range((cap - base * n_chunks) // 128):
        chunks[i] += 128
    return idx, counts, tuple(chunks), cap, sel


_CACHED = {}


def kernel(**inputs) -> np.ndarray:
    inp = {k: np.asarray(v) for k, v in inputs.items()}
    V1r, V2r, V3r, C1, C2, C3 = _fold_params(inp)
    x = inp['x'].astype(np.float32)
    WgT = np.ascontiguousarray(inp['Wg'].T.astype(np.float32))

    # ---- stage A: gate + shared expert (data-parallel over tokens) ----
    if "A" not in _CACHED:
        _CACHED["A"] = _build_kernel_A()
    ncA = _CACHED["A"]
    shA = dict(WgT=WgT, V1s=V1r[8], V2s=V2r[8], V3s=V3r[8],
               C1s=C1[8], C2s=C2[8], C3s=C3[8])
    mapsA = []
    for c in range(N_CORES):
        xT = np.ascontiguousarray(x[c * TOK:(c + 1) * TOK].T)
        m = dict(shA)
        m['xTr'] = xT
        m['xT32'] = xT
        mapsA.append(m)
    resA = run_bass_kernel_spmd(ncA, mapsA, core_ids=list(range(N_CORES)))
    wsum = np.concatenate([r["wsum"] for r in resA.results], axis=0)
    sharedA = [r["shared"] for r in resA.results]

    # ---- host dispatch: gather tokens per expert ----
    idx, counts, chunks, cap, sel = _route_from_wsum(wsum)

    # ---- stage B: expert-parallel FFN on gathered tokens ----
    if _CACHED.get("B_chunks") != chunks:
        _CACHED["B"] = _build_kernel_B(chunks)
        _CACHED["B_chunks"] = chunks
    ncB = _CACHED["B"]
    mapsB = []
    for e in range(E):
        cnt = counts[e]
        xg = np.zeros((D, cap), np.float32)
        xg[:, :cnt] = x[idx[e]].T
        wrow = np.zeros((cap,), np.float32)
        wrow[:cnt] = wsum[idx[e], e]
        mapsB.append(dict(xg=xg, wrow=wrow, V1s=V1r[e], V2s=V2r[e], V3s=V3r[e],
                          C1s=C1[e], C2s=C2[e], C3s=C3[e]))
    resB = run_bass_kernel_spmd(ncB, mapsB, core_ids=list(range(N_CORES)))
    outbs = [r["outb"] for r in resB.results]

    # ---- host combine alignment: scatter contributions back by token ----
    first_e = np.argmax(sel, axis=1)
    cont1 = np.zeros((N_TOKENS, O), np.float32)
    cont2 = np.zeros((N_TOKENS, O), np.float32)
    for e in range(E):
        toks = idx[e]
        outT = np.ascontiguousarray(outbs[e][:, :counts[e]].T)
        is_first = first_e[toks] == e
        cont1[toks[is_first]] = outT[is_first]
        cont2[toks[~is_first]] = outT[~is_first]

    # ---- stage C: final on-device sum ----
    if "C" not in _CACHED:
        _CACHED["C"] = _build_kernel_C()
    ncC = _CACHED["C"]
    mapsC = []
    for c in range(N_CORES):
        sl = slice(c * TOK, (c + 1) * TOK)
        mapsC.append(dict(sharedT=np.ascontiguousarray(sharedA[c].T),
                          cont1=cont1[sl], cont2=cont2[sl]))
    resC = run_bass_kernel_spmd(ncC, mapsC, core_ids=list(range(N_CORES)))
    out = np.concatenate([r["out"] for r in resC.results], axis=0)

    _CACHED["timing"] = [(ncA, mapsA), (ncB, mapsB), (ncC, mapsC)]
    return out.astype(np.float32)


# revision 4
# speedup vs baseline: 110.8691x; 1.0975x over previous
"""DeepseekMoE Trainium2 kernel — routed 3-stage pipeline on 8 NeuronCores.

Stage A (data-parallel, 1024 tokens/core): gate computed with a true-fp32
  matmul (so top-2 selection matches the fp32 reference) producing the
  normalized top-2 combine weights, plus the shared-expert FFN.
Stage B (expert-parallel, one expert per core): 3-layer FFN over the tokens
  routed to that expert (host-gathered to a runtime-sized capacity), with
  the per-token combine weight applied on device.
Stage C (data-parallel): out = shared + contrib1 + contrib2 on device.

Expert matmuls run in float32r (fp22 multiply, fp32 accumulate). Eval-mode
BatchNorm is folded into the expert weights host-side (pure parameter
preprocessing). Host code between stages only moves data (gather/scatter by
the device-computed top-2 indices); all per-token arithmetic is on device.
"""
import numpy as np
import concourse.mybir as mybir
import concourse.tile as tile
from concourse import bacc
from concourse.bass_utils import run_bass_kernel_spmd

F32 = mybir.dt.float32
F32R = mybir.dt.float32r

N_TOKENS, D, H, O, E = 8192, 1024, 2048, 1024, 8
KD, KH, MH, MO = D // 128, H // 128, H // 128, O // 128
NEXP = 9  # 8 routed experts + shared (index 8)
EPS = 1e-5
BIG = 1e30
N_CORES = 8
TOK = N_TOKENS // N_CORES
Relu = mybir.ActivationFunctionType.Relu
Sigm = mybir.ActivationFunctionType.Sigmoid
Expf = mybir.ActivationFunctionType.Exp


# ---------------------------------------------------------------- host prep
def _fold_params(inp):
    """Fold eval-mode BN into the expert weights (host-side, O(weights))."""
    def tiles_kxm(V, KT, MT):
        return np.ascontiguousarray(
            V.reshape(KT, 128, MT, 128).transpose(2, 1, 0, 3))

    V1s, V2s, V3s, C1s, C2s, C3s = [], [], [], [], [], []
    for e in range(NEXP):
        if e < E:
            W1, b1 = inp['W1'][e], inp['b1'][e]
            g1, be1, m1, v1 = inp['g1'][e], inp['be1'][e], inp['m1'][e], inp['v1'][e]
            W2, b2 = inp['W2'][e], inp['b2'][e]
            g2, be2, m2, v2 = inp['g2'][e], inp['be2'][e], inp['m2'][e], inp['v2'][e]
            W3, b3 = inp['W3'][e], inp['b3'][e]
        else:
            W1, b1 = inp['sW1'], inp['sb1']
            g1, be1, m1, v1 = inp['sg1'], inp['sbe1'], inp['sm1'], inp['sv1']
            W2, b2 = inp['sW2'], inp['sb2']
            g2, be2, m2, v2 = inp['sg2'], inp['sbe2'], inp['sm2'], inp['sv2']
            W3, b3 = inp['sW3'], inp['sb3']
        s1 = g1 / np.sqrt(v1 + EPS); t1 = be1 - m1 * s1
        s2 = g2 / np.sqrt(v2 + EPS); t2 = be2 - m2 * s2
        V1 = W1.T.astype(np.float32)
        c1 = b1.astype(np.float32)
        V2 = (s1[:, None] * W2.T).astype(np.float32)
        c2 = (b2 + t1 @ W2.T).astype(np.float32)
        V3 = (s2[:, None] * W3.T).astype(np.float32)
        c3 = (b3 + t2 @ W3.T).astype(np.float32)
        V1s.append(tiles_kxm(V1, KD, MH))
        V2s.append(tiles_kxm(V2, KH, MH))
        V3s.append(tiles_kxm(V3, KH, MO))
        C1s.append(np.ascontiguousarray(c1.reshape(MH, 128).T))
        C2s.append(np.ascontiguousarray(c2.reshape(MH, 128).T))
        C3s.append(np.ascontiguousarray(c3.reshape(MO, 128).T))
    return (np.stack(V1s), np.stack(V2s), np.stack(V3s),
            np.stack(C1s), np.stack(C2s), np.stack(C3s))


# ------------------------------------------------------------ kernel builders
def _ffn3(nc, pools, xg, V1_ap, V2_ap, V3_ap, c1_sb, c2_sb, c3_sb, ntok, emit):
    """Feature-major 3-layer FFN on `ntok` tokens (multiple of 128).
    xg: SBUF [128, KD, ntok] f32r. emit(mi, nsl, psum) consumes L3 psum."""
    wpool, ps, apool = pools["w"], pools["ps"], pools["act"]
    nsls = []
    s = 0
    while ntok - s > 0:
        rest = ntok - s
        if rest > 512 and rest < 768:
            # avoid a <256 tail: f32r matmuls below 256 free-dim run at 1/4 rate
            w = rest - 256
        else:
            w = min(512, rest)
        nsls.append(slice(s, s + w))
        s += w
    a1 = apool.tile([128, KH, ntok], F32R, tag="a1", name="a1")
    for mi in range(MH):
        wt = wpool.tile([128, KD, 128], F32R, tag="w", name="wt1")
        nc.sync.dma_start(wt, V1_ap[mi])
        for nsl in nsls:
            nn = nsl.stop - nsl.start
            pp = ps.tile([128, 512], F32, tag="ps", name="pp1")[:, :nn]
            for ki in range(KD):
                nc.tensor.matmul(pp, wt[:, ki], xg[:, ki, nsl],
                                 start=(ki == 0), stop=(ki == KD - 1))
            nc.scalar.activation(a1[:, mi, nsl], pp, Relu,
                                 bias=c1_sb[:, mi:mi + 1], scale=1.0)
    a2 = apool.tile([128, KH, ntok], F32R, tag="a2", name="a2")
    for mi in range(MH):
        wta = wpool.tile([128, KD, 128], F32R, tag="w", name="wta")
        nc.sync.dma_start(wta, V2_ap[mi, :, :KD])
        wtb = wpool.tile([128, KD, 128], F32R, tag="w", name="wtb")
        nc.sync.dma_start(wtb, V2_ap[mi, :, KD:])
        for nsl in nsls:
            nn = nsl.stop - nsl.start
            pp = ps.tile([128, 512], F32, tag="ps", name="pp2")[:, :nn]
            for ki in range(KH):
                wt = wta if ki < KD else wtb
                nc.tensor.matmul(pp, wt[:, ki % KD], a1[:, ki, nsl],
                                 start=(ki == 0), stop=(ki == KH - 1))
            nc.scalar.activation(a2[:, mi, nsl], pp, Relu,
                                 bias=c2_sb[:, mi:mi + 1], scale=1.0)
    for mi in range(MO):
        wta = wpool.tile([128, KD, 128], F32R, tag="w", name="wta3")
        nc.sync.dma_start(wta, V3_ap[mi, :, :KD])
        wtb = wpool.tile([128, KD, 128], F32R, tag="w", name="wtb3")
        nc.sync.dma_start(wtb, V3_ap[mi, :, KD:])
        for nsl in nsls:
            nn = nsl.stop - nsl.start
            pp = ps.tile([128, 512], F32, tag="ps", name="pp3")[:, :nn]
            for ki in range(KH):
                wt = wta if ki < KD else wtb
                nc.tensor.matmul(pp, wt[:, ki % KD], a2[:, ki, nsl],
                                 start=(ki == 0), stop=(ki == KH - 1))
            emit(mi, nsl, pp)


def _build_kernel_A():
    """Gate (true fp32) + shared expert. Outputs wsum [TOK, E], shared [O, TOK]."""
    nc = bacc.Bacc("TRN2", target_bir_lowering=False, debug=False,
                   num_devices=N_CORES)
    xTr_d = nc.dram_tensor("xTr", [D, TOK], F32R, kind="ExternalInput")
    xT32_d = nc.dram_tensor("xT32", [D, TOK], F32, kind="ExternalInput")
    wg_d = nc.dram_tensor("WgT", [D, E], F32, kind="ExternalInput")
    V1_d = nc.dram_tensor("V1s", [MH, 128, KD, 128], F32R, kind="ExternalInput")
    V2_d = nc.dram_tensor("V2s", [MH, 128, KH, 128], F32R, kind="ExternalInput")
    V3_d = nc.dram_tensor("V3s", [MO, 128, KH, 128], F32R, kind="ExternalInput")
    C1_d = nc.dram_tensor("C1s", [128, MH], F32, kind="ExternalInput")
    C2_d = nc.dram_tensor("C2s", [128, MH], F32, kind="ExternalInput")
    C3_d = nc.dram_tensor("C3s", [128, MO], F32, kind="ExternalInput")
    wsum_d = nc.dram_tensor("wsum", [TOK, E], F32, kind="ExternalOutput")
    sh_d = nc.dram_tensor("shared", [O, TOK], F32, kind="ExternalOutput")

    TT = TOK // 128
    with tile.TileContext(nc) as tc:
        with tc.tile_pool(name="const", bufs=1) as cpool, \
             tc.tile_pool(name="acts", bufs=1) as apool, \
             tc.tile_pool(name="wts", bufs=3) as wpool, \
             tc.tile_pool(name="bias", bufs=1) as bpool, \
             tc.tile_pool(name="tmp", bufs=3) as tpool, \
             tc.tile_pool(name="gate", bufs=2) as gpool, \
             tc.tile_pool(name="ps", bufs=8, space="PSUM") as ps:
            xTr_sb = cpool.tile([128, KD, TOK], F32R)
            nc.sync.dma_start(xTr_sb, xTr_d.ap().rearrange("(k p) t -> p k t", p=128))
            wg_sb = cpool.tile([128, KD, E], F32)
            nc.sync.dma_start(wg_sb, wg_d.ap().rearrange("(k p) e -> p k e", p=128))
            # xT32 (gate-only) shares its slot with a2 (FFN L2+)
            xT32_sb = apool.tile([128, KD, TOK], F32, tag="a2", name="xT32_sb")
            nc.sync.dma_start(xT32_sb, xT32_d.ap().rearrange("(k p) t -> p k t", p=128))

            for ti in range(TT):
                tsl = slice(ti * 128, (ti + 1) * 128)
                pg = ps.tile([128, 512], F32, tag="ps", name="pg")[:, :E]
                for ki in range(KD):
                    nc.tensor.matmul(pg, xT32_sb[:, ki, tsl], wg_sb[:, ki],
                                     start=(ki == 0), stop=(ki == KD - 1))
                s = gpool.tile([128, E], F32)
                nc.vector.tensor_copy(s, pg)
                m1 = gpool.tile([128, 1], F32)
                nc.vector.tensor_reduce(m1, s, axis=mybir.AxisListType.X,
                                        op=mybir.AluOpType.max)
                nm1 = gpool.tile([128, 1], F32)
                nc.vector.tensor_scalar_mul(nm1, m1, -1.0)
                msk = gpool.tile([128, E], F32)
                nc.vector.tensor_tensor(msk, s, m1.to_broadcast((128, E)),
                                        op=mybir.AluOpType.is_equal)
                nc.vector.tensor_scalar_mul(msk, msk, -BIG)
                nc.vector.tensor_tensor(msk, s, msk, op=mybir.AluOpType.add)
                m2 = gpool.tile([128, 1], F32)
                nc.vector.tensor_reduce(m2, msk, axis=mybir.AxisListType.X,
                                        op=mybir.AluOpType.max)
                r = gpool.tile([128, E], F32)
                nc.scalar.activation(r, s, Expf, bias=nm1, scale=1.0)
                e2 = gpool.tile([128, 1], F32)
                nc.scalar.activation(e2, m2, Expf, bias=nm1, scale=1.0)
                den = gpool.tile([128, 1], F32)
                nc.vector.tensor_scalar_add(den, e2, 1.0)
                rec = gpool.tile([128, 1], F32)
                nc.vector.reciprocal(rec, den)
                ge = gpool.tile([128, E], F32)
                nc.vector.tensor_tensor(ge, s, m2.to_broadcast((128, E)),
                                        op=mybir.AluOpType.is_ge)
                w = gpool.tile([128, E], F32)
                nc.vector.tensor_tensor(w, r, ge, op=mybir.AluOpType.mult)
                nc.vector.tensor_scalar_mul(w, w, rec)
                nc.sync.dma_start(wsum_d.ap()[tsl], w)

            c1_sb = bpool.tile([128, MH], F32, name="c1_sb")
            nc.sync.dma_start(c1_sb, C1_d.ap())
            c2_sb = bpool.tile([128, MH], F32, name="c2_sb")
            nc.sync.dma_start(c2_sb, C2_d.ap())
            c3_sb = bpool.tile([128, MO], F32, name="c3_sb")
            nc.sync.dma_start(c3_sb, C3_d.ap())

            def emit(mi, nsl, pp):
                nn = nsl.stop - nsl.start
                sg = tpool.tile([128, 512], F32, name="sg")[:, :nn]
                nc.scalar.activation(sg, pp, Sigm,
                                     bias=c3_sb[:, mi:mi + 1], scale=1.0)
                nc.sync.dma_start(sh_d.ap()[mi * 128:(mi + 1) * 128, nsl], sg)

            pools = {"w": wpool, "ps": ps, "act": apool}
            _ffn3(nc, pools, xTr_sb, V1_d.ap(), V2_d.ap(), V3_d.ap(),
                  c1_sb, c2_sb, c3_sb, TOK, emit)
    nc.compile()
    return nc


def _build_kernel_B(chunks):
    """One expert per core on gathered tokens; output pre-weighted [O, cap]."""
    C = sum(chunks)
    nc = bacc.Bacc("TRN2", target_bir_lowering=False, debug=False,
                   num_devices=N_CORES)
    xg_d = nc.dram_tensor("xg", [D, C], F32R, kind="ExternalInput")
    wrow_d = nc.dram_tensor("wrow", [C], F32, kind="ExternalInput")
    V1_d = nc.dram_tensor("V1s", [MH, 128, KD, 128], F32R, kind="ExternalInput")
    V2_d = nc.dram_tensor("V2s", [MH, 128, KH, 128], F32R, kind="ExternalInput")
    V3_d = nc.dram_tensor("V3s", [MO, 128, KH, 128], F32R, kind="ExternalInput")
    C1_d = nc.dram_tensor("C1s", [128, MH], F32, kind="ExternalInput")
    C2_d = nc.dram_tensor("C2s", [128, MH], F32, kind="ExternalInput")
    C3_d = nc.dram_tensor("C3s", [128, MO], F32, kind="ExternalInput")
    outb_d = nc.dram_tensor("outb", [O, C], F32, kind="ExternalOutput")

    with tile.TileContext(nc) as tc:
        with tc.tile_pool(name="xgp", bufs=1) as xgpool, \
             tc.tile_pool(name="acts", bufs=1) as apool, \
             tc.tile_pool(name="wts", bufs=4) as wpool, \
             tc.tile_pool(name="bias", bufs=1) as bpool, \
             tc.tile_pool(name="wb", bufs=1) as wbpool, \
             tc.tile_pool(name="tmp", bufs=3) as tpool, \
             tc.tile_pool(name="ps", bufs=8, space="PSUM") as ps:
            c1_sb = bpool.tile([128, MH], F32, name="c1_sb")
            nc.sync.dma_start(c1_sb, C1_d.ap())
            c2_sb = bpool.tile([128, MH], F32, name="c2_sb")
            nc.sync.dma_start(c2_sb, C2_d.ap())
            c3_sb = bpool.tile([128, MO], F32, name="c3_sb")
            nc.sync.dma_start(c3_sb, C3_d.ap())
            pools = {"w": wpool, "ps": ps, "act": apool}

            off = 0
            mx = max(chunks)
            for ch in chunks:
                xg = xgpool.tile([128, KD, mx], F32R, tag="xg", name="xg")
                nc.sync.dma_start(
                    xg[:, :, :ch],
                    xg_d.ap().rearrange("(k p) t -> p k t", p=128)[:, :, off:off + ch])
                wbc = wbpool.tile([128, mx], F32, tag="wbc", name="wbc")
                nc.sync.dma_start(
                    wbc[:, :ch],
                    wrow_d.ap()[None, off:off + ch].to_broadcast((128, ch)))

                def emit(mi, nsl, pp, off=off, wbc=wbc):
                    nn = nsl.stop - nsl.start
                    sg = tpool.tile([128, 512], F32, name="sg")[:, :nn]
                    nc.scalar.activation(sg, pp, Sigm,
                                         bias=c3_sb[:, mi:mi + 1], scale=1.0)
                    nc.vector.tensor_tensor(sg, sg, wbc[:, nsl],
                                            op=mybir.AluOpType.mult)
                    nc.sync.dma_start(
                        outb_d.ap()[mi * 128:(mi + 1) * 128,
                                    off + nsl.start:off + nsl.stop], sg)

                _ffn3(nc, pools, xg[:, :, :ch], V1_d.ap(), V2_d.ap(),
                      V3_d.ap(), c1_sb, c2_sb, c3_sb, ch, emit)
                off += ch
    nc.compile()
    return nc


def _build_kernel_C():
    """out = sharedT + cont1 + cont2, all token-major [TOK, O]."""
    nc = bacc.Bacc("TRN2", target_bir_lowering=False, debug=False,
                   num_devices=N_CORES)
    sh_d = nc.dram_tensor("sharedT", [TOK, O], F32, kind="ExternalInput")
    c1_d = nc.dram_tensor("cont1", [TOK, O], F32, kind="ExternalInput")
    c2_d = nc.dram_tensor("cont2", [TOK, O], F32, kind="ExternalInput")
    out_d = nc.dram_tensor("out", [TOK, O], F32, kind="ExternalOutput")
    with tile.TileContext(nc) as tc:
        with tc.tile_pool(name="sb", bufs=3) as sb:
            for ti in range(TOK // 128):
                tsl = slice(ti * 128, (ti + 1) * 128)
                a = sb.tile([128, O], F32, name="a")
                nc.sync.dma_start(a, sh_d.ap()[tsl])
                b = sb.tile([128, O], F32, name="b")
                nc.sync.dma_start(b, c1_d.ap()[tsl])
                c = sb.tile([128, O], F32, name="c")
                nc.sync.dma_start(c, c2_d.ap()[tsl])
                nc.vector.tensor_tensor(a, a, b, op=mybir.AluOpType.add)
                nc.vector.tensor_tensor(a, a, c, op=mybir.AluOpType.add)
                nc.sync.dma_start(out_d.ap()[tsl], a)
    nc.compile()
    return nc


# ------------------------------------------------------------------ host glue
def _route_from_wsum(wsum):
    """Top-2 experts per token from the device-computed combine weights."""
    n = wsum.shape[0]
    top2 = np.argpartition(-wsum, 2, axis=1)[:, :2]
    sel = np.zeros_like(wsum, dtype=bool)
    sel[np.arange(n)[:, None], top2] = True
    idx = [np.nonzero(sel[:, e])[0] for e in range(E)]
    counts = np.array([len(i) for i in idx])
    cap = int(np.ceil(counts.max() / 128) * 128)
    n_chunks = max(1, -(-cap // 1152))
    base = cap // n_chunks // 128 * 128
    chunks = [base] * n_chunks
    for i in range((cap - base * n_chunks) // 128):
        chunks[i] += 128
    return idx, counts, tuple(chunks), cap, sel


_CACHED = {}


def kernel(**inputs) -> np.ndarray:
    inp = {k: np.asarray(v) for k, v in inputs.items()}
    V1r, V2r, V3r, C1, C2, C3 = _fold_params(inp)
    x = inp['x'].astype(np.float32)
    WgT = np.ascontiguousarray(inp['Wg'].T.astype(np.float32))

    # ---- stage A: gate + shared expert (data-parallel over tokens) ----
    if "A" not in _CACHED:
        _CACHED["A"] = _build_kernel_A()
    ncA = _CACHED["A"]
    shA = dict(WgT=WgT, V1s=V1r[8], V2s=V2r[8], V3s=V3r[8],
               C1s=C1[8], C2s=C2[8], C3s=C3[8])
    mapsA = []
    for c in range(N_CORES):
        xT = np.ascontiguousarray(x[c * TOK:(c + 1) * TOK].T)
        m = dict(shA)
        m['xTr'] = xT
        m['xT32'] = xT
        mapsA.append(m)
    resA = run_bass_kernel_spmd(ncA, mapsA, core_ids=list(range(N_CORES)))
    wsum = np.concatenate([r["wsum"] for r in resA.results], axis=0)
    sharedA = [r["shared"] for r in resA.results]

    # ---- host dispatch: gather tokens per expert ----
    idx, counts, chunks, cap, sel = _route_from_wsum(wsum)

    # ---- stage B: expert-parallel FFN on gathered tokens ----
    if _CACHED.get("B_chunks") != chunks:
        _CACHED["B"] = _build_kernel_B(chunks)
        _CACHED["B_chunks"] = chunks
    ncB = _CACHED["B"]
    mapsB = []
    for e in range(E):
        cnt = counts[e]
        xg = np.zeros((D, cap), np.float32)
        xg[:, :cnt] = x[idx[e]].T
        wrow = np.zeros((cap,), np.float32)
        wrow[:cnt] = wsum[idx[e], e]
        mapsB.append(dict(xg=xg, wrow=wrow, V1s=V1r[e], V2s=V2r[e], V3s=V3r[e],
                          C1s=C1[e], C2s=C2[e], C3s=C3[e]))
    resB = run_bass_kernel_spmd(ncB, mapsB, core_ids=list(range(N_CORES)))
    outbs = [r["outb"] for r in resB.results]

    # ---- host combine alignment: scatter contributions back by token ----
    first_e = np.argmax(sel, axis=1)
    cont1 = np.zeros((N_TOKENS, O), np.float32)
    cont2 = np.zeros((N_TOKENS, O), np.float32)
    for e in range(E):
        toks = idx[e]
        outT = np.ascontiguousarray(outbs[e][:, :counts[e]].T)
        is_first = first_e[toks] == e
        cont1[toks[is_first]] = outT[is_first]
        cont2[toks[~is_first]] = outT[~is_first]

    # ---- stage C: final on-device sum ----
    if "C" not in _CACHED:
        _CACHED["C"] = _build_kernel_C()
    ncC = _CACHED["C"]
    mapsC = []
    for c in range(N_CORES):
        sl = slice(c * TOK, (c + 1) * TOK)
        mapsC.append(dict(sharedT=np.ascontiguousarray(sharedA[c].T),
                          cont1=cont1[sl], cont2=cont2[sl]))
    resC = run_bass_kernel_spmd(ncC, mapsC, core_ids=list(range(N_CORES)))
    out = np.concatenate([r["out"] for r in resC.results], axis=0)

    _CACHED["timing"] = [(ncA, mapsA), (ncB, mapsB), (ncC, mapsC)]
    return out.astype(np.float32)


# revision 5
# speedup vs baseline: 111.5828x; 1.0064x over previous
"""DeepseekMoE Trainium2 kernel — routed 3-stage pipeline on 8 NeuronCores.

Stage A (data-parallel, 1024 tokens/core): gate computed with a true-fp32
  matmul (so top-2 selection matches the fp32 reference) producing the
  normalized top-2 combine weights, plus the shared-expert FFN.
Stage B (expert-parallel, one expert per core): 3-layer FFN over the tokens
  routed to that expert (host-gathered to a runtime-sized capacity), with
  the per-token combine weight applied on device.
Stage C (data-parallel): out = shared + contrib1 + contrib2 on device.

Expert matmuls run in float32r (fp22 multiply, fp32 accumulate). Eval-mode
BatchNorm is folded into the expert weights host-side (pure parameter
preprocessing). Host code between stages only moves data (gather/scatter by
the device-computed top-2 indices); all per-token arithmetic is on device.
"""
import numpy as np
import concourse.mybir as mybir
import concourse.tile as tile
from concourse import bacc
from concourse.bass_utils import run_bass_kernel_spmd

F32 = mybir.dt.float32
F32R = mybir.dt.float32r

N_TOKENS, D, H, O, E = 8192, 1024, 2048, 1024, 8
KD, KH, MH, MO = D // 128, H // 128, H // 128, O // 128
NEXP = 9  # 8 routed experts + shared (index 8)
EPS = 1e-5
BIG = 1e30
N_CORES = 8
TOK = N_TOKENS // N_CORES
Relu = mybir.ActivationFunctionType.Relu
Sigm = mybir.ActivationFunctionType.Sigmoid
Expf = mybir.ActivationFunctionType.Exp


# ---------------------------------------------------------------- host prep
def _fold_params(inp):
    """Fold eval-mode BN into the expert weights (host-side, O(weights))."""
    def tiles_kxm(V, KT, MT):
        return np.ascontiguousarray(
            V.reshape(KT, 128, MT, 128).transpose(2, 1, 0, 3))

    V1s, V2s, V3s, C1s, C2s, C3s = [], [], [], [], [], []
    for e in range(NEXP):
        if e < E:
            W1, b1 = inp['W1'][e], inp['b1'][e]
            g1, be1, m1, v1 = inp['g1'][e], inp['be1'][e], inp['m1'][e], inp['v1'][e]
            W2, b2 = inp['W2'][e], inp['b2'][e]
            g2, be2, m2, v2 = inp['g2'][e], inp['be2'][e], inp['m2'][e], inp['v2'][e]
            W3, b3 = inp['W3'][e], inp['b3'][e]
        else:
            W1, b1 = inp['sW1'], inp['sb1']
            g1, be1, m1, v1 = inp['sg1'], inp['sbe1'], inp['sm1'], inp['sv1']
            W2, b2 = inp['sW2'], inp['sb2']
            g2, be2, m2, v2 = inp['sg2'], inp['sbe2'], inp['sm2'], inp['sv2']
            W3, b3 = inp['sW3'], inp['sb3']
        s1 = g1 / np.sqrt(v1 + EPS); t1 = be1 - m1 * s1
        s2 = g2 / np.sqrt(v2 + EPS); t2 = be2 - m2 * s2
        V1 = W1.T.astype(np.float32)
        c1 = b1.astype(np.float32)
        V2 = (s1[:, None] * W2.T).astype(np.float32)
        c2 = (b2 + t1 @ W2.T).astype(np.float32)
        V3 = (s2[:, None] * W3.T).astype(np.float32)
        c3 = (b3 + t2 @ W3.T).astype(np.float32)
        V1s.append(tiles_kxm(V1, KD, MH))
        V2s.append(tiles_kxm(V2, KH, MH))
        V3s.append(tiles_kxm(V3, KH, MO))
        C1s.append(np.ascontiguousarray(c1.reshape(MH, 128).T))
        C2s.append(np.ascontiguousarray(c2.reshape(MH, 128).T))
        C3s.append(np.ascontiguousarray(c3.reshape(MO, 128).T))
    return (np.stack(V1s), np.stack(V2s), np.stack(V3s),
            np.stack(C1s), np.stack(C2s), np.stack(C3s))


# ------------------------------------------------------------ kernel builders
def _ffn3(nc, pools, xg, V1_ap, V2_ap, V3_ap, c1_sb, c2_sb, c3_sb, ntok, emit):
    """Feature-major 3-layer FFN on `ntok` tokens (multiple of 128).
    xg: SBUF [128, KD, ntok] f32r. emit(mi, nsl, psum) consumes L3 psum."""
    wpool, ps, apool = pools["w"], pools["ps"], pools["act"]
    nsls = []
    s = 0
    while ntok - s > 0:
        rest = ntok - s
        if rest > 512 and rest < 768:
            # avoid a <256 tail: f32r matmuls below 256 free-dim run at 1/4 rate
            w = rest - 256
        else:
            w = min(512, rest)
        nsls.append(slice(s, s + w))
        s += w
    a1 = apool.tile([128, KH, ntok], F32R, tag="a1", name="a1")
    for mi in range(MH):
        wt = wpool.tile([128, KD, 128], F32R, tag="w", name="wt1")
        nc.sync.dma_start(wt, V1_ap[mi])
        for nsl in nsls:
            nn = nsl.stop - nsl.start
            pp = ps.tile([128, 512], F32, tag="ps", name="pp1")[:, :nn]
            for ki in range(KD):
                nc.tensor.matmul(pp, wt[:, ki], xg[:, ki, nsl],
                                 start=(ki == 0), stop=(ki == KD - 1))
            nc.scalar.activation(a1[:, mi, nsl], pp, Relu,
                                 bias=c1_sb[:, mi:mi + 1], scale=1.0)
    a2 = apool.tile([128, KH, ntok], F32R, tag="a2", name="a2")
    for mi in range(MH):
        wta = wpool.tile([128, KD, 128], F32R, tag="w", name="wta")
        nc.sync.dma_start(wta, V2_ap[mi, :, :KD])
        wtb = wpool.tile([128, KD, 128], F32R, tag="w", name="wtb")
        nc.sync.dma_start(wtb, V2_ap[mi, :, KD:])
        for nsl in nsls:
            nn = nsl.stop - nsl.start
            pp = ps.tile([128, 512], F32, tag="ps", name="pp2")[:, :nn]
            for ki in range(KH):
                wt = wta if ki < KD else wtb
                nc.tensor.matmul(pp, wt[:, ki % KD], a1[:, ki, nsl],
                                 start=(ki == 0), stop=(ki == KH - 1))
            nc.scalar.activation(a2[:, mi, nsl], pp, Relu,
                                 bias=c2_sb[:, mi:mi + 1], scale=1.0)
    for mi in range(MO):
        wta = wpool.tile([128, KD, 128], F32R, tag="w", name="wta3")
        nc.sync.dma_start(wta, V3_ap[mi, :, :KD])
        wtb = wpool.tile([128, KD, 128], F32R, tag="w", name="wtb3")
        nc.sync.dma_start(wtb, V3_ap[mi, :, KD:])
        for nsl in nsls:
            nn = nsl.stop - nsl.start
            pp = ps.tile([128, 512], F32, tag="ps", name="pp3")[:, :nn]
            for ki in range(KH):
                wt = wta if ki < KD else wtb
                nc.tensor.matmul(pp, wt[:, ki % KD], a2[:, ki, nsl],
                                 start=(ki == 0), stop=(ki == KH - 1))
            emit(mi, nsl, pp)


def _build_kernel_A():
    """Gate (true fp32) + shared expert. Outputs wsum [TOK, E], shared [O, TOK]."""
    nc = bacc.Bacc("TRN2", target_bir_lowering=False, debug=False,
                   num_devices=N_CORES)
    xTr_d = nc.dram_tensor("xTr", [D, TOK], F32R, kind="ExternalInput")
    xT32_d = nc.dram_tensor("xT32", [D, TOK], F32, kind="ExternalInput")
    wg_d = nc.dram_tensor("WgT", [D, E], F32, kind="ExternalInput")
    V1_d = nc.dram_tensor("V1s", [MH, 128, KD, 128], F32R, kind="ExternalInput")
    V2_d = nc.dram_tensor("V2s", [MH, 128, KH, 128], F32R, kind="ExternalInput")
    V3_d = nc.dram_tensor("V3s", [MO, 128, KH, 128], F32R, kind="ExternalInput")
    C1_d = nc.dram_tensor("C1s", [128, MH], F32, kind="ExternalInput")
    C2_d = nc.dram_tensor("C2s", [128, MH], F32, kind="ExternalInput")
    C3_d = nc.dram_tensor("C3s", [128, MO], F32, kind="ExternalInput")
    wsum_d = nc.dram_tensor("wsum", [TOK, E], F32, kind="ExternalOutput")
    sh_d = nc.dram_tensor("shared", [O, TOK], F32, kind="ExternalOutput")

    TT = TOK // 128
    with tile.TileContext(nc) as tc:
        with tc.tile_pool(name="const", bufs=1) as cpool, \
             tc.tile_pool(name="acts", bufs=1) as apool, \
             tc.tile_pool(name="wts", bufs=3) as wpool, \
             tc.tile_pool(name="bias", bufs=1) as bpool, \
             tc.tile_pool(name="tmp", bufs=3) as tpool, \
             tc.tile_pool(name="gate", bufs=2) as gpool, \
             tc.tile_pool(name="ps", bufs=8, space="PSUM") as ps:
            xTr_sb = cpool.tile([128, KD, TOK], F32R)
            for _ki in range(KD):
                nc.sync.dma_start(xTr_sb[:, _ki], xTr_d.ap().rearrange(
                    "(k p) t -> p k t", p=128)[:, _ki])
            wg_sb = cpool.tile([128, KD, E], F32)
            nc.sync.dma_start(wg_sb, wg_d.ap().rearrange("(k p) e -> p k e", p=128))
            # xT32 (gate-only) shares its slot with a2 (FFN L2+)
            xT32_sb = apool.tile([128, KD, TOK], F32, tag="a2", name="xT32_sb")
            for _ki in range(KD):
                nc.sync.dma_start(xT32_sb[:, _ki], xT32_d.ap().rearrange(
                    "(k p) t -> p k t", p=128)[:, _ki])

            for ti in range(TT):
                tsl = slice(ti * 128, (ti + 1) * 128)
                pg = ps.tile([128, 512], F32, tag="ps", name="pg")[:, :E]
                for ki in range(KD):
                    nc.tensor.matmul(pg, xT32_sb[:, ki, tsl], wg_sb[:, ki],
                                     start=(ki == 0), stop=(ki == KD - 1))
                s = gpool.tile([128, E], F32)
                nc.vector.tensor_copy(s, pg)
                m1 = gpool.tile([128, 1], F32)
                nc.vector.tensor_reduce(m1, s, axis=mybir.AxisListType.X,
                                        op=mybir.AluOpType.max)
                nm1 = gpool.tile([128, 1], F32)
                nc.vector.tensor_scalar_mul(nm1, m1, -1.0)
                msk = gpool.tile([128, E], F32)
                nc.vector.tensor_tensor(msk, s, m1.to_broadcast((128, E)),
                                        op=mybir.AluOpType.is_equal)
                nc.vector.tensor_scalar_mul(msk, msk, -BIG)
                nc.vector.tensor_tensor(msk, s, msk, op=mybir.AluOpType.add)
                m2 = gpool.tile([128, 1], F32)
                nc.vector.tensor_reduce(m2, msk, axis=mybir.AxisListType.X,
                                        op=mybir.AluOpType.max)
                r = gpool.tile([128, E], F32)
                nc.scalar.activation(r, s, Expf, bias=nm1, scale=1.0)
                e2 = gpool.tile([128, 1], F32)
                nc.scalar.activation(e2, m2, Expf, bias=nm1, scale=1.0)
                den = gpool.tile([128, 1], F32)
                nc.vector.tensor_scalar_add(den, e2, 1.0)
                rec = gpool.tile([128, 1], F32)
                nc.vector.reciprocal(rec, den)
                ge = gpool.tile([128, E], F32)
                nc.vector.tensor_tensor(ge, s, m2.to_broadcast((128, E)),
                                        op=mybir.AluOpType.is_ge)
                w = gpool.tile([128, E], F32)
                nc.vector.tensor_tensor(w, r, ge, op=mybir.AluOpType.mult)
                nc.vector.tensor_scalar_mul(w, w, rec)
                nc.sync.dma_start(wsum_d.ap()[tsl], w)

            c1_sb = bpool.tile([128, MH], F32, name="c1_sb")
            nc.sync.dma_start(c1_sb, C1_d.ap())
            c2_sb = bpool.tile([128, MH], F32, name="c2_sb")
            nc.sync.dma_start(c2_sb, C2_d.ap())
            c3_sb = bpool.tile([128, MO], F32, name="c3_sb")
            nc.sync.dma_start(c3_sb, C3_d.ap())

            def emit(mi, nsl, pp):
                nn = nsl.stop - nsl.start
                sg = tpool.tile([128, 512], F32, name="sg")[:, :nn]
                nc.scalar.activation(sg, pp, Sigm,
                                     bias=c3_sb[:, mi:mi + 1], scale=1.0)
                nc.sync.dma_start(sh_d.ap()[mi * 128:(mi + 1) * 128, nsl], sg)

            pools = {"w": wpool, "ps": ps, "act": apool}
            _ffn3(nc, pools, xTr_sb, V1_d.ap(), V2_d.ap(), V3_d.ap(),
                  c1_sb, c2_sb, c3_sb, TOK, emit)
    nc.compile()
    return nc


def _build_kernel_B(chunks):
    """One expert per core on gathered tokens; output pre-weighted [O, cap]."""
    C = sum(chunks)
    nc = bacc.Bacc("TRN2", target_bir_lowering=False, debug=False,
                   num_devices=N_CORES)
    xg_d = nc.dram_tensor("xg", [D, C], F32R, kind="ExternalInput")
    wrow_d = nc.dram_tensor("wrow", [C], F32, kind="ExternalInput")
    V1_d = nc.dram_tensor("V1s", [MH, 128, KD, 128], F32R, kind="ExternalInput")
    V2_d = nc.dram_tensor("V2s", [MH, 128, KH, 128], F32R, kind="ExternalInput")
    V3_d = nc.dram_tensor("V3s", [MO, 128, KH, 128], F32R, kind="ExternalInput")
    C1_d = nc.dram_tensor("C1s", [128, MH], F32, kind="ExternalInput")
    C2_d = nc.dram_tensor("C2s", [128, MH], F32, kind="ExternalInput")
    C3_d = nc.dram_tensor("C3s", [128, MO], F32, kind="ExternalInput")
    outb_d = nc.dram_tensor("outb", [O, C], F32, kind="ExternalOutput")

    with tile.TileContext(nc) as tc:
        with tc.tile_pool(name="xgp", bufs=1) as xgpool, \
             tc.tile_pool(name="acts", bufs=1) as apool, \
             tc.tile_pool(name="wts", bufs=4) as wpool, \
             tc.tile_pool(name="bias", bufs=1) as bpool, \
             tc.tile_pool(name="wb", bufs=1) as wbpool, \
             tc.tile_pool(name="tmp", bufs=3) as tpool, \
             tc.tile_pool(name="ps", bufs=8, space="PSUM") as ps:
            c1_sb = bpool.tile([128, MH], F32, name="c1_sb")
            nc.sync.dma_start(c1_sb, C1_d.ap())
            c2_sb = bpool.tile([128, MH], F32, name="c2_sb")
            nc.sync.dma_start(c2_sb, C2_d.ap())
            c3_sb = bpool.tile([128, MO], F32, name="c3_sb")
            nc.sync.dma_start(c3_sb, C3_d.ap())
            pools = {"w": wpool, "ps": ps, "act": apool}

            off = 0
            mx = max(chunks)
            for ch in chunks:
                xg = xgpool.tile([128, KD, mx], F32R, tag="xg", name="xg")
                for _ki in range(KD):
                    nc.sync.dma_start(
                        xg[:, _ki, :ch],
                        xg_d.ap().rearrange("(k p) t -> p k t",
                                            p=128)[:, _ki, off:off + ch])
                wbc = wbpool.tile([128, mx], F32, tag="wbc", name="wbc")
                nc.sync.dma_start(
                    wbc[:, :ch],
                    wrow_d.ap()[None, off:off + ch].to_broadcast((128, ch)))

                def emit(mi, nsl, pp, off=off, wbc=wbc):
                    nn = nsl.stop - nsl.start
                    sg = tpool.tile([128, 512], F32, name="sg")[:, :nn]
                    nc.scalar.activation(sg, pp, Sigm,
                                         bias=c3_sb[:, mi:mi + 1], scale=1.0)
                    nc.vector.tensor_tensor(sg, sg, wbc[:, nsl],
                                            op=mybir.AluOpType.mult)
                    nc.sync.dma_start(
                        outb_d.ap()[mi * 128:(mi + 1) * 128,
                                    off + nsl.start:off + nsl.stop], sg)

                _ffn3(nc, pools, xg[:, :, :ch], V1_d.ap(), V2_d.ap(),
                      V3_d.ap(), c1_sb, c2_sb, c3_sb, ch, emit)
                off += ch
    nc.compile()
    return nc


def _build_kernel_C():
    """out = sharedT + cont1 + cont2, all token-major [TOK, O]."""
    nc = bacc.Bacc("TRN2", target_bir_lowering=False, debug=False,
                   num_devices=N_CORES)
    sh_d = nc.dram_tensor("sharedT", [TOK, O], F32, kind="ExternalInput")
    c1_d = nc.dram_tensor("cont1", [TOK, O], F32, kind="ExternalInput")
    c2_d = nc.dram_tensor("cont2", [TOK, O], F32, kind="ExternalInput")
    out_d = nc.dram_tensor("out", [TOK, O], F32, kind="ExternalOutput")
    with tile.TileContext(nc) as tc:
        with tc.tile_pool(name="sb", bufs=3) as sb:
            for ti in range(TOK // 128):
                tsl = slice(ti * 128, (ti + 1) * 128)
                a = sb.tile([128, O], F32, name="a")
                nc.sync.dma_start(a, sh_d.ap()[tsl])
                b = sb.tile([128, O], F32, name="b")
                nc.sync.dma_start(b, c1_d.ap()[tsl])
                c = sb.tile([128, O], F32, name="c")
                nc.sync.dma_start(c, c2_d.ap()[tsl])
                nc.vector.tensor_tensor(a, a, b, op=mybir.AluOpType.add)
                nc.vector.tensor_tensor(a, a, c, op=mybir.AluOpType.add)
                nc.sync.dma_start(out_d.ap()[tsl], a)
    nc.compile()
    return nc


# ------------------------------------------------------------------ host glue
def _route_from_wsum(wsum):
    """Top-2 experts per token from the device-computed combine weights."""
    n = wsum.shape[0]
    top2 = np.argpartition(-wsum, 2, axis=1)[:, :2]
    sel = np.zeros_like(wsum, dtype=bool)
    sel[np.arange(n)[:, None], top2] = True
    idx = [np.nonzero(sel[:, e])[0] for e in range(E)]
    counts = np.array([len(i) for i in idx])
    cap = int(np.ceil(counts.max() / 128) * 128)
    n_chunks = max(1, -(-cap // 1152))
    base = cap // n_chunks // 128 * 128
    chunks = [base] * n_chunks
    for i in range((cap - base * n_chunks) // 128):
        chunks[i] += 128
    return idx, counts, tuple(chunks), cap, sel


_CACHED = {}


def kernel(**inputs) -> np.ndarray:
    inp = {k: np.asarray(v) for k, v in inputs.items()}
    V1r, V2r, V3r, C1, C2, C3 = _fold_params(inp)
    x = inp['x'].astype(np.float32)
    WgT = np.ascontiguousarray(inp['Wg'].T.astype(np.float32))

    # ---- stage A: gate + shared expert (data-parallel over tokens) ----
    if "A" not in _CACHED:
        _CACHED["A"] = _build_kernel_A()
    ncA = _CACHED["A"]
    shA = dict(WgT=WgT, V1s=V1r[8], V2s=V2r[8], V3s=V3r[8],
               C1s=C1[8], C2s=C2[8], C3s=C3[8])
    mapsA = []
    for c in range(N_CORES):
        xT = np.ascontiguousarray(x[c * TOK:(c + 1) * TOK].T)
        m = dict(shA)
        m['xTr'] = xT
        m['xT32'] = xT
        mapsA.append(m)
    resA = run_bass_kernel_spmd(ncA, mapsA, core_ids=list(range(N_CORES)))
    wsum = np.concatenate([r["wsum"] for r in resA.results], axis=0)
    sharedA = [r["shared"] for r in resA.results]

    # ---- host dispatch: gather tokens per expert ----
    idx, counts, chunks, cap, sel = _route_from_wsum(wsum)

    # ---- stage B: expert-parallel FFN on gathered tokens ----
    if _CACHED.get("B_chunks") != chunks:
        _CACHED["B"] = _build_kernel_B(chunks)
        _CACHED["B_chunks"] = chunks
    ncB = _CACHED["B"]
    mapsB = []
    for e in range(E):
        cnt = counts[e]
        xg = np.zeros((D, cap), np.float32)
        xg[:, :cnt] = x[idx[e]].T
        wrow = np.zeros((cap,), np.float32)
        wrow[:cnt] = wsum[idx[e], e]
        mapsB.append(dict(xg=xg, wrow=wrow, V1s=V1r[e], V2s=V2r[e], V3s=V3r[e],
                          C1s=C1[e], C2s=C2[e], C3s=C3[e]))
    resB = run_bass_kernel_spmd(ncB, mapsB, core_ids=list(range(N_CORES)))
    outbs = [r["outb"] for r in resB.results]

    # ---- host combine alignment: scatter contributions back by token ----
    first_e = np.argmax(sel, axis=1)
    cont1 = np.zeros((N_TOKENS, O), np.float32)
    cont2 = np.zeros((N_TOKENS, O), np.float32)
    for e in range(E):
        toks = idx[e]
        outT = np.ascontiguousarray(outbs[e][:, :counts[e]].T)
        is_first = first_e[toks] == e
        cont1[toks[is_first]] = outT[is_first]
        cont2[toks[~is_first]] = outT[~is_first]

    # ---- stage C: final on-device sum ----
    if "C" not in _CACHED:
        _CACHED["C"] = _build_kernel_C()
    ncC = _CACHED["C"]
    mapsC = []
    for c in range(N_CORES):
        sl = slice(c * TOK, (c + 1) * TOK)
        mapsC.append(dict(sharedT=np.ascontiguousarray(sharedA[c].T),
                          cont1=cont1[sl], cont2=cont2[sl]))
    resC = run_bass_kernel_spmd(ncC, mapsC, core_ids=list(range(N_CORES)))
    out = np.concatenate([r["out"] for r in resC.results], axis=0)

    _CACHED["timing"] = [(ncA, mapsA), (ncB, mapsB), (ncC, mapsC)]
    return out.astype(np.float32)
